# revision 30
# baseline (speedup 1.0000x reference)
"""Distributed causal attention with RoPE for trn2 (8 NeuronCores).

Problem: B=2, S=2048, DIM=2048, H=16 heads, D=128.
  out = softmax(causal(rope(xq) @ rope(xk)^T / sqrt(D))) @ xv @ wo^T

Sharding: one batch per 4-core group, 4 heads per core:
  core c: batch bb=c//4, heads 4*(c%4)..4*(c%4)+3, output columns
  (c%4)*512..+512 of its batch.  Attention is fully local; the only
  collective is an AllGather within each 4-core group per sequence
  chunk (each core contributes its 4 heads' attention output, gathers
  the full 2048-dim hidden), after which each core computes its
  512-column slice of the output projection for its batch.

Pipeline over the 4 sequence chunks:
  proj(t) -> attention(t) -> AllGather(t) -> out-projection(t-2)

Key layout/engine choices (vs the v1 kernel):
  - host pre-permutes all DRAM operands to [128, tile, cols] so every
    SBUF load is ONE DMA descriptor chain (sync-engine issue cost was
    a bottleneck: 342 DMAs -> ~80)
  - softmax row-sums: exp tiles are accumulated elementwise on the DVE
    (bf16, 2x mode) and reduced across partitions with a single ones-
    matmul per (chunk, head) -- removes 160 rowsum matmuls (~38us PE)
  - causal masking multiplies only the 128x128 diagonal block (single
    triangular mask tile) on the DVE.  NOT on gpsimd: the Pool cores
    execute collective data movement, and any work there slows the
    AllGathers 3-4x
  - reciprocal_approx_fast instead of reciprocal (4us -> ~0.8us each)
  - RoPE runs on bf16 SBUF copies (ACT does the PSUM->SBUF cast), so
    the DVE multiplies get the 2x 16-bit mode
  - PSUM: pp/pr/po shared tag x2, pk(scores) x3, pu(PV) x3 = 8 banks
  - the framework tracks collectives with ONE cumulative semaphore:
    reads of gather t's output must be issued BEFORE collective t+1 is
    triggered, or they wait on t+1; the last chunk's gather is split
    per head so it overlaps the remaining attention, and its SBUF
    loads are batched around the drain out-projections so the sync
    queue never head-of-line blocks on a collective wait
  - output stored bf16 (host casts back) to halve the final DMA drain
"""

import math
import sys

sys.path.insert(0, "/opt/trn_rl_repo")

import numpy as np
import ml_dtypes

import concourse.bass as bass
import concourse.mybir as mybir
import concourse.tile as tile
from concourse import bacc
from concourse.bass_utils import run_bass_kernel_spmd

BF16 = mybir.dt.bfloat16
F32 = mybir.dt.float32

B, S, DIM, H, D = 2, 2048, 2048, 16, 128
NCORES = 8
HPC = 4                  # heads per core (one batch per core)
SBW = 512                # sequence chunk width
NSB = S // SBW           # 4
NKT = DIM // 128         # 16 contraction tiles for projections
OSL = 512                # output column slice per core
HD = HPC * D             # local hidden slice = 512
GROUPS = [[0, 1, 2, 3], [4, 5, 6, 7]]
INV_SQRT_D = 1.0 / math.sqrt(D)
SWAP_MASK = [i + 1 if i % 2 == 0 else i - 1 for i in range(32)]
MULT = mybir.AluOpType.mult
ADD = mybir.AluOpType.add

LAST_RESULT = None
_CACHED_NC = None


def _rope_chain(nc, sb, j, xt, w_t, dst, cos_t, sin_t, ta, psP, nm):
    """One head-slice projection + RoPE.  dst is a [128, SBW] bf16 view."""
    ssl = slice(sb * SBW, (sb + 1) * SBW)
    pp = psP.tile([128, SBW], F32, tag="pq", bufs=2, name=f"pp{nm}")
    for i in range(NKT):
        nc.tensor.matmul(
            pp[:], lhsT=w_t[:, i, j * 128:(j + 1) * 128], rhs=xt[:, i, :],
            start=(i == 0), stop=(i == NKT - 1))
    tq = ta.tile([128, SBW], BF16, tag="tq", bufs=2, name=f"tq{nm}")
    nc.scalar.copy(tq[:], pp[:])
    swq = ta.tile([128, SBW], BF16, tag="sw", bufs=2, name=f"sw{nm}")
    nc.vector.stream_shuffle(swq[:], tq[:], SWAP_MASK)
    nc.vector.tensor_tensor(out=dst, in0=tq[:], in1=cos_t[:, ssl], op=MULT)
    m2 = ta.tile([128, SBW], BF16, tag="m2", bufs=1, name=f"m2{nm}")
    nc.vector.tensor_tensor(out=m2[:], in0=swq[:], in1=sin_t[:, ssl], op=MULT)
    nc.vector.tensor_tensor(out=dst, in0=dst, in1=m2[:], op=ADD)


def _proj_block(nc, sb, xt, wq_t, wk_t, wv_t, cos_t, sin_t,
                qTc, kT_t, v_t, ta, psP):
    """QKV projections (+RoPE on Q,K) for sequence chunk sb, 4 heads."""
    ssl = slice(sb * SBW, (sb + 1) * SBW)
    for j in range(HPC):
        _rope_chain(nc, sb, j, xt, wk_t, kT_t[:, j, ssl], cos_t, sin_t,
                    ta, psP, f"k{sb}{j}")
    for j in range(HPC):
        _rope_chain(nc, sb, j, xt, wq_t, qTc[:, j, :], cos_t, sin_t,
                    ta, psP, f"q{sb}{j}")
    for m in range(4):
        pv = psP.tile([128, HD], F32, tag="pq", bufs=2, name=f"pv{sb}{m}")
        for i in range(NKT):
            nc.tensor.matmul(
                pv[:], lhsT=xt[:, i, m * 128:(m + 1) * 128],
                rhs=wv_t[:, i, :], start=(i == 0), stop=(i == NKT - 1))
        nc.scalar.copy(v_t[:, sb * 4 + m, :], pv[:])


def _att_chunk(nc, qb, qTc, kT_t, v_t, ao_c, msk_t, ones_m,
               eb, ac, tb, psP, on_finish, depth=4):
    """Flash-style attention for one q-chunk, all 4 heads, as ONE
    software-pipelined stream.

    k-tiles stream through: scores -> exp (ACT) -> diagonal mask (DVE)
    -> elementwise row-sum accumulation (DVE, bf16 2x) + PV matmul.
    A single pend queue spans all 4 head instances, so one instance's
    tail consumes interleave with the next instance's scores instead of
    draining against the exp->mask chain (~0.9us stall per instance
    otherwise).  Each instance's reduction (one ones-matmul across
    partitions + reciprocal + normalize) fires when its last k-tile is
    consumed; on_finish(hi) is then invoked for per-instance staging.
    """
    nkt = 4 * qb + 4
    pend = []

    def finish(hi, Eacc, pu):
        pr = psP.tile([128, 512], F32, tag="pq", bufs=2, name=f"pr{qb}{hi}")
        nc.tensor.matmul(pr[:], lhsT=ones_m[:], rhs=Eacc[:],
                         start=True, stop=True)
        rinv = tb.tile([128, 512], F32, tag="ri", bufs=1, name=f"ri{qb}{hi}")
        nc.vector.reciprocal_approx_fast(out=rinv[:], in_=pr[:])
        nc.vector.tensor_tensor(
            out=ao_c[:, hi, :], in0=pu[:], in1=rinv[:], op=MULT)
        on_finish(hi)

    def make_consume(hi, Eacc, pu):
        def consume(kt, et, c0):
            if kt == 0:
                nc.vector.tensor_copy(out=Eacc[:], in_=et[:])
            else:
                nc.vector.tensor_tensor(
                    out=Eacc[:, c0:], in0=Eacc[:, c0:], in1=et[:, c0:],
                    op=ADD)
            nc.tensor.matmul(
                pu[:, c0:], lhsT=v_t[:, kt, hi * 128:(hi + 1) * 128],
                rhs=et[:, c0:], start=(kt == 0), stop=(kt == nkt - 1))
            if kt == nkt - 1:
                finish(hi, Eacc, pu)
        return consume

    def drain_one():
        consume, kt, et, c0 = pend.pop(0)
        consume(kt, et, c0)

    for hi in range(HPC):
        Eacc = ac.tile([128, 512], BF16, tag="ea", bufs=2, name=f"ea{qb}{hi}")
        pu = psP.tile([128, 512], F32, tag="pu", bufs=3, name=f"pu{qb}{hi}")
        consume = make_consume(hi, Eacc, pu)
        for kt in range(nkt):
            dj = kt - 4 * qb
            c0 = max(dj, 0) * 128
            pk = psP.tile([128, 512], F32, tag="pk", bufs=3,
                          name=f"pk{qb}{hi}{kt}")
            nc.tensor.matmul(
                pk[:, c0:], lhsT=kT_t[:, hi, kt * 128:(kt + 1) * 128],
                rhs=qTc[:, hi, c0:], start=True, stop=True)
            et = eb.tile([128, 512], BF16, tag="e", bufs=depth + 1,
                         name=f"et{qb}{hi}{kt}")
            nc.scalar.activation(
                et[:, c0:], pk[:, c0:], mybir.ActivationFunctionType.Exp,
                scale=INV_SQRT_D)
            if dj >= 0:
                # only the 128x128 diagonal block is partially masked
                nc.vector.tensor_tensor(
                    out=et[:, c0:c0 + 128], in0=et[:, c0:c0 + 128],
                    in1=msk_t[:], op=MULT)
            pend.append((consume, kt, et, c0))
            if len(pend) > depth:
                drain_one()
    while pend:
        drain_one()


def _out_proj_block(nc, qb, agt, wo_t, out, tco, psP):
    """Output projection for s-chunk qb from the gathered heads."""
    for st in range(4):
        po = psP.tile([128, OSL], F32, tag="pq", bufs=2, name=f"po{qb}{st}")
        for i in range(NKT):
            nc.tensor.matmul(
                po[:], lhsT=agt[:, i, st * 128:(st + 1) * 128],
                rhs=wo_t[:, i, :], start=(i == 0), stop=(i == NKT - 1))
        ot = tco.tile([128, OSL], BF16, tag="ot", bufs=4, name=f"ot{qb}{st}")
        nc.scalar.copy(ot[:], po[:])
        r0 = qb * 512 + st * 128
        nc.sync.dma_start(out=out[r0:r0 + 128, :], in_=ot[:])


def _build():
    nc = bacc.Bacc("TRN2", target_bir_lowering=False, debug=False,
                   num_devices=NCORES)

    # all DRAM operands pre-permuted host-side to [128, tile, cols]
    xT = nc.declare_dram_parameter("xT", [128, NKT, S], BF16, isOutput=False)
    wqT = nc.declare_dram_parameter("wqT", [128, NKT, HD], BF16, isOutput=False)
    wkT = nc.declare_dram_parameter("wkT", [128, NKT, HD], BF16, isOutput=False)
    wvT = nc.declare_dram_parameter("wvT", [128, NKT, HD], BF16, isOutput=False)
    woT = nc.declare_dram_parameter("woT", [128, NKT, OSL], BF16, isOutput=False)
    cosb = nc.declare_dram_parameter("cosb", [128, S], BF16, isOutput=False)
    sinb = nc.declare_dram_parameter("sinb", [128, S], BF16, isOutput=False)
    msk = nc.declare_dram_parameter("msk", [128, 128], BF16, isOutput=False)
    out = nc.declare_dram_parameter("out", [S, OSL], BF16, isOutput=True)

    with tile.TileContext(nc) as tc:
        with (
            tc.tile_pool(name="res", bufs=1) as res,
            tc.tile_pool(name="xa", bufs=1) as xa,
            tc.tile_pool(name="qa", bufs=1) as qa,
            tc.tile_pool(name="ta", bufs=1) as ta,
            tc.tile_pool(name="eb", bufs=1) as eb,
            tc.tile_pool(name="ac", bufs=1) as ac,
            tc.tile_pool(name="tb", bufs=1) as tb,
            tc.tile_pool(name="xc", bufs=1) as xc,
            tc.tile_pool(name="tco", bufs=1) as tco,
            tc.tile_pool(name="dram", bufs=1, space="DRAM") as dram,
            tc.tile_pool(name="psP", bufs=1, space="PSUM") as psP,
        ):
            # ---- resident tiles -------------------------------------------
            msk_t = res.tile([128, 128], BF16)
            ones_m = res.tile([128, 128], BF16)
            kT_t = res.tile([128, HPC, S], BF16)     # rope'd K^T per head
            v_t = res.tile([128, NKT, HD], BF16)     # V natural [s, dv]
            wq_t = res.tile([128, NKT, HD], BF16)
            wk_t = res.tile([128, NKT, HD], BF16)
            wv_t = res.tile([128, NKT, HD], BF16)
            wo_t = res.tile([128, NKT, OSL], BF16)
            cos_t = res.tile([128, S], BF16)
            sin_t = res.tile([128, S], BF16)

            xt_tiles = {
                t: xa.tile([128, NKT, SBW], BF16, tag="xt", bufs=2,
                           name=f"xt{t}")
                for t in range(NSB)
            }

            # prologue DMAs, in first-use order; fine 2-tile pieces so the
            # first K chain starts streaming as soon as possible.  cos/sin
            # are needed by the first rope (~16us in); wq by the Q chains
            # (~25us) -- split into pieces so the single large transfers
            # don't gate them.
            for g in range(2):
                gsl = slice(g, g + 1)
                nc.sync.dma_start(out=wk_t[:, gsl, :], in_=wkT[:, gsl, :])
                nc.sync.dma_start(out=xt_tiles[0][:, gsl, :],
                                  in_=xT[:, gsl, 0:SBW])
            nc.sync.dma_start(out=cos_t[:], in_=cosb[:])
            nc.sync.dma_start(out=sin_t[:], in_=sinb[:])
            for g in range(1, 8):
                gsl = slice(g * 2, (g + 1) * 2)
                nc.sync.dma_start(out=wk_t[:, gsl, :], in_=wkT[:, gsl, :])
                nc.sync.dma_start(out=xt_tiles[0][:, gsl, :],
                                  in_=xT[:, gsl, 0:SBW])
            for g in range(4):
                gsl = slice(g * 4, (g + 1) * 4)
                nc.sync.dma_start(out=wq_t[:, gsl, :], in_=wqT[:, gsl, :])
            nc.sync.dma_start(out=msk_t[:], in_=msk[:])
            for g in range(2):
                gsl = slice(g * 8, (g + 1) * 8)
                nc.sync.dma_start(out=wv_t[:, gsl, :], in_=wvT[:, gsl, :])
            nc.sync.dma_start(out=wo_t[:], in_=woT[:])
            nc.vector.memset(ones_m[:], 1.0)

            def stage(t, ao_c):
                ag_in = dram.tile([128, HPC, SBW], BF16, tag="agi", bufs=2,
                                  name=f"agi{t}")
                nc.sync.dma_start(out=ag_in[:], in_=ao_c[:])
                ag_out = dram.tile([4, 128, HPC, SBW], BF16, tag="ago",
                                   bufs=2, name=f"ago{t}")
                nc.gpsimd.collective_compute(
                    "AllGather",
                    mybir.AluOpType.bypass,
                    ins=[ag_in.opt()],
                    outs=[ag_out.opt()],
                    replica_groups=GROUPS,
                )
                return ag_out

            def stage_hi(t, hi, ao_c):
                # per-instance gather: lets the last chunk's collective
                # start while attention on the remaining heads still runs
                ag_in = dram.tile([128, SBW], BF16, tag="agih", bufs=4,
                                  name=f"agih{t}{hi}")
                nc.sync.dma_start(out=ag_in[:], in_=ao_c[:, hi, :])
                ag_out = dram.tile([4, 128, SBW], BF16, tag="agoh", bufs=4,
                                   name=f"agoh{t}{hi}")
                nc.gpsimd.collective_compute(
                    "AllGather",
                    mybir.AluOpType.bypass,
                    ins=[ag_in.opt()],
                    outs=[ag_out.opt()],
                    replica_groups=GROUPS,
                )
                return ag_out

            def load_agt(t, ag_out):
                agt = xc.tile([128, NKT, SBW], BF16, tag="agt", bufs=2,
                              name=f"agt{t}")
                # one DMA per source rank: 4 parallel queues (a single
                # transposed descriptor chain serializes 2MB through one
                # queue and stalls the out-projection ~10us)
                for r in range(4):
                    nc.sync.dma_start(out=agt[:, r * 4:(r + 1) * 4, :],
                                      in_=ag_out[r])
                return agt

            # ---- pipeline over the 4 sequence chunks ----------------------
            # NOTE on ordering: the framework tracks all collectives with one
            # cumulative semaphore, so any ag_out read issued AFTER cc(t) in
            # program order waits for cc(t) to land.  All load_agt calls for
            # already-landed collectives must therefore be issued BEFORE the
            # next collective_compute.
            ago_map = {}
            agt_map = {}
            for t in range(NSB):
                qTc = qa.tile([128, HPC, SBW], BF16, tag="qT", bufs=2,
                              name=f"qT{t}")
                _proj_block(nc, t, xt_tiles[t], wq_t, wk_t, wv_t,
                            cos_t, sin_t, qTc, kT_t, v_t, ta, psP)
                if t < NSB - 1:
                    nc.sync.dma_start(
                        out=xt_tiles[t + 1][:],
                        in_=xT[:, :, (t + 1) * SBW:(t + 2) * SBW])
                ao_c = qa.tile([128, HPC, SBW], BF16, tag="ao", bufs=2,
                               name=f"ao{t}")
                if t >= 2:
                    agt_map[t - 2] = load_agt(t - 2, ago_map[t - 2])
                if t == NSB - 1:
                    # pre-issue before any cc(3,hi) so it only waits cc(2)
                    agt_map[t - 1] = load_agt(t - 1, ago_map[t - 1])
                    # reuses xt(2)'s SBUF (free after proj(2)) so the
                    # interleaved loads below never wait on out-proj
                    agt3 = xa.tile([128, NKT, SBW], BF16, tag="xt", bufs=2,
                                   name="agt3")

                    def load_agt3(hi, ago_hi):
                        # head hi owns dim-tile slots hi, hi+4, hi+8, hi+12
                        nc.sync.dma_start(
                            out=agt3[:, hi:NKT:4, :],
                            in_=ago_hi[:].transpose([1, 0, 2]))

                    ago3 = []
                    _att_chunk(nc, t, qTc, kT_t, v_t, ao_c, msk_t, ones_m,
                               eb, ac, tb, psP,
                               on_finish=lambda hi: ago3.append(
                                   stage_hi(t, hi, ao_c)))
                    # issue before op(1)'s out-DMAs so the loads land as
                    # soon as cc(3,h3) completes; tco bufs=4 absorbs the
                    # brief sync-queue block while they wait
                    for hi in range(HPC):
                        load_agt3(hi, ago3[hi])
                else:
                    _att_chunk(nc, t, qTc, kT_t, v_t, ao_c, msk_t, ones_m,
                               eb, ac, tb, psP, on_finish=lambda hi: None)
                    ago_map[t] = stage(t, ao_c)
                if t >= 2:
                    _out_proj_block(nc, t - 2, agt_map[t - 2], wo_t, out,
                                    tco, psP)
            _out_proj_block(nc, NSB - 2, agt_map[NSB - 2], wo_t, out, tco,
                            psP)
            _out_proj_block(nc, NSB - 1, agt3, wo_t, out, tco, psP)
    nc.compile()
    return nc


def _host_prep(x, wq, wk, wv, wo):
    """Build per-core input maps (host-side permutes + bf16 casts)."""
    bf = ml_dtypes.bfloat16

    def perm(a2d):
        # [DIM(rows=2048), C] -> [128, 16, C] with rows = i*128 + p
        c = a2d.shape[1]
        return np.ascontiguousarray(
            a2d.reshape(NKT, 128, c).transpose(1, 0, 2)).astype(bf)

    inv = 1.0 / (10000.0 ** (np.arange(0, D, 2, dtype=np.float64) / D))
    ang = np.outer(np.arange(S, dtype=np.float64), inv)         # [S, 64]
    cos = np.cos(ang).T
    sin = np.sin(ang).T
    cosb = np.repeat(cos, 2, axis=0).astype(np.float32)         # [128, S]
    sinb = np.repeat(sin, 2, axis=0).astype(np.float32)
    sinb[0::2, :] *= -1.0    # even d rows: -sin ; odd rows: +sin

    ki = np.arange(128)[:, None]
    cj = np.arange(128)[None, :]
    mskb = (ki <= cj).astype(np.float32).astype(bf)             # [128, 128]
    cosb, sinb = cosb.astype(bf), sinb.astype(bf)

    xT_b = [perm(np.ascontiguousarray(x[b].T)) for b in range(B)]

    in_maps = []
    for c in range(NCORES):
        bb, ci = c // 4, c % 4
        hrows = slice(ci * HD, (ci + 1) * HD)       # this core's 4 heads
        ocols = slice(ci * OSL, (ci + 1) * OSL)     # its output columns
        in_maps.append({
            "xT": xT_b[bb],
            "wqT": perm(np.ascontiguousarray(wq[hrows].T)),
            "wkT": perm(np.ascontiguousarray(wk[hrows].T)),
            "wvT": perm(np.ascontiguousarray(wv[hrows].T)),
            "woT": perm(np.ascontiguousarray(wo[ocols, :].T)),
            "cosb": cosb,
            "sinb": sinb,
            "msk": mskb,
        })
    return in_maps


def kernel(x, wq, wk, wv, wo):
    global LAST_RESULT, _CACHED_NC
    if _CACHED_NC is None:
        _CACHED_NC = _build()
    nc = _CACHED_NC
    in_maps = _host_prep(x, wq, wk, wv, wo)
    res = run_bass_kernel_spmd(nc, in_maps, core_ids=list(range(NCORES)))
    LAST_RESULT = res
    out = np.empty((B, S, DIM), np.float32)
    for c in range(NCORES):
        bb, ci = c // 4, c % 4
        out[bb, :, ci * OSL:(ci + 1) * OSL] = res.results[c]["out"].astype(np.float32)
    return out


# revision 31
# speedup vs baseline: 1.0105x; 1.0105x over previous
"""Distributed causal attention with RoPE for trn2 (8 NeuronCores).

Problem: B=2, S=2048, DIM=2048, H=16 heads, D=128.
  out = softmax(causal(rope(xq) @ rope(xk)^T / sqrt(D))) @ xv @ wo^T

Sharding: one batch per 4-core group, 4 heads per core:
  core c: batch bb=c//4, heads 4*(c%4)..4*(c%4)+3, output columns
  (c%4)*512..+512 of its batch.  Attention is fully local; the only
  collective is an AllGather within each 4-core group per sequence
  chunk (each core contributes its 4 heads' attention output, gathers
  the full 2048-dim hidden), after which each core computes its
  512-column slice of the output projection for its batch.

Pipeline over the 4 sequence chunks:
  proj(t) -> attention(t) -> AllGather(t) -> out-projection(t-2)

Key layout/engine choices (vs the v1 kernel):
  - host pre-permutes all DRAM operands to [128, tile, cols] so every
    SBUF load is ONE DMA descriptor chain (sync-engine issue cost was
    a bottleneck: 342 DMAs -> ~80)
  - softmax row-sums: exp tiles are accumulated elementwise on the DVE
    (bf16, 2x mode) and reduced across partitions with a single ones-
    matmul per (chunk, head) -- removes 160 rowsum matmuls (~38us PE)
  - causal masking multiplies only the 128x128 diagonal block (single
    triangular mask tile) on the DVE.  NOT on gpsimd: the Pool cores
    execute collective data movement, and any work there slows the
    AllGathers 3-4x
  - reciprocal_approx_fast instead of reciprocal (4us -> ~0.8us each)
  - RoPE runs on bf16 SBUF copies (ACT does the PSUM->SBUF cast), so
    the DVE multiplies get the 2x 16-bit mode
  - PSUM: pp/pr/po shared tag x2, pk(scores) x3, pu(PV) x3 = 8 banks
  - the framework tracks collectives with ONE cumulative semaphore:
    reads of gather t's output must be issued BEFORE collective t+1 is
    triggered, or they wait on t+1; the last chunk's gather is split
    per head so it overlaps the remaining attention, and its SBUF
    loads are batched around the drain out-projections so the sync
    queue never head-of-line blocks on a collective wait
  - output stored bf16 (host casts back) to halve the final DMA drain
"""

import math
import sys

sys.path.insert(0, "/opt/trn_rl_repo")

import numpy as np
import ml_dtypes

import concourse.bass as bass
import concourse.mybir as mybir
import concourse.tile as tile
from concourse import bacc
from concourse.bass_utils import run_bass_kernel_spmd

BF16 = mybir.dt.bfloat16
F32 = mybir.dt.float32

B, S, DIM, H, D = 2, 2048, 2048, 16, 128
NCORES = 8
HPC = 4                  # heads per core (one batch per core)
SBW = 512                # sequence chunk width
NSB = S // SBW           # 4
NKT = DIM // 128         # 16 contraction tiles for projections
OSL = 512                # output column slice per core
HD = HPC * D             # local hidden slice = 512
GROUPS = [[0, 1, 2, 3], [4, 5, 6, 7]]
INV_SQRT_D = 1.0 / math.sqrt(D)
SWAP_MASK = [i + 1 if i % 2 == 0 else i - 1 for i in range(32)]
MULT = mybir.AluOpType.mult
ADD = mybir.AluOpType.add

LAST_RESULT = None
_CACHED_NC = None


def _rope_chain(nc, sb, j, xt, w_t, dst, cos_t, sin_t, ta, psP, nm):
    """One head-slice projection + RoPE.  dst is a [128, SBW] bf16 view."""
    ssl = slice(sb * SBW, (sb + 1) * SBW)
    pp = psP.tile([128, SBW], F32, tag="pq", bufs=2, name=f"pp{nm}")
    for i in range(NKT):
        nc.tensor.matmul(
            pp[:], lhsT=w_t[:, i, j * 128:(j + 1) * 128], rhs=xt[:, i, :],
            start=(i == 0), stop=(i == NKT - 1))
    tq = ta.tile([128, SBW], BF16, tag="tq", bufs=2, name=f"tq{nm}")
    nc.scalar.copy(tq[:], pp[:])
    swq = ta.tile([128, SBW], BF16, tag="sw", bufs=2, name=f"sw{nm}")
    nc.vector.stream_shuffle(swq[:], tq[:], SWAP_MASK)
    nc.vector.tensor_tensor(out=dst, in0=tq[:], in1=cos_t[:, ssl], op=MULT)
    m2 = ta.tile([128, SBW], BF16, tag="m2", bufs=1, name=f"m2{nm}")
    nc.vector.tensor_tensor(out=m2[:], in0=swq[:], in1=sin_t[:, ssl], op=MULT)
    nc.vector.tensor_tensor(out=dst, in0=dst, in1=m2[:], op=ADD)


def _proj_block(nc, sb, xt, wq_t, wk_t, wv_t, cos_t, sin_t,
                qTc, kT_t, v_t, ta, psP):
    """QKV projections (+RoPE on Q,K) for sequence chunk sb, 4 heads."""
    ssl = slice(sb * SBW, (sb + 1) * SBW)
    for j in range(HPC):
        _rope_chain(nc, sb, j, xt, wk_t, kT_t[:, j, ssl], cos_t, sin_t,
                    ta, psP, f"k{sb}{j}")
    for j in range(HPC):
        _rope_chain(nc, sb, j, xt, wq_t, qTc[:, j, :], cos_t, sin_t,
                    ta, psP, f"q{sb}{j}")
    for m in range(4):
        pv = psP.tile([128, HD], F32, tag="pq", bufs=2, name=f"pv{sb}{m}")
        for i in range(NKT):
            nc.tensor.matmul(
                pv[:], lhsT=xt[:, i, m * 128:(m + 1) * 128],
                rhs=wv_t[:, i, :], start=(i == 0), stop=(i == NKT - 1))
        nc.scalar.copy(v_t[:, sb * 4 + m, :], pv[:])


def _att_chunk(nc, qb, qTc, kT_t, v_t, ao_c, msk_t, ones_m,
               eb, ac, tb, psP, on_finish, depth=4):
    """Flash-style attention for one q-chunk, all 4 heads, as ONE
    software-pipelined stream.

    k-tiles stream through: scores -> exp (ACT) -> diagonal mask (DVE)
    -> elementwise row-sum accumulation (DVE, bf16 2x) + PV matmul.
    A single pend queue spans all 4 head instances, so one instance's
    tail consumes interleave with the next instance's scores instead of
    draining against the exp->mask chain (~0.9us stall per instance
    otherwise).  Each instance's reduction (one ones-matmul across
    partitions + reciprocal + normalize) fires when its last k-tile is
    consumed; on_finish(hi) is then invoked for per-instance staging.
    """
    nkt = 4 * qb + 4
    pend = []

    def finish(hi, Eacc, pu):
        pr = psP.tile([128, 512], F32, tag="pq", bufs=2, name=f"pr{qb}{hi}")
        nc.tensor.matmul(pr[:], lhsT=ones_m[:], rhs=Eacc[:],
                         start=True, stop=True)
        rinv = tb.tile([128, 512], F32, tag="ri", bufs=1, name=f"ri{qb}{hi}")
        nc.vector.reciprocal_approx_fast(out=rinv[:], in_=pr[:])
        nc.vector.tensor_tensor(
            out=ao_c[:, hi, :], in0=pu[:], in1=rinv[:], op=MULT)
        on_finish(hi)

    def make_consume(hi, Eacc, pu):
        def consume(kt, et, c0):
            if kt == 0:
                nc.vector.tensor_copy(out=Eacc[:], in_=et[:])
            else:
                nc.vector.tensor_tensor(
                    out=Eacc[:, c0:], in0=Eacc[:, c0:], in1=et[:, c0:],
                    op=ADD)
            nc.tensor.matmul(
                pu[:, c0:], lhsT=v_t[:, kt, hi * 128:(hi + 1) * 128],
                rhs=et[:, c0:], start=(kt == 0), stop=(kt == nkt - 1))
            if kt == nkt - 1:
                finish(hi, Eacc, pu)
        return consume

    def drain_one():
        consume, kt, et, c0 = pend.pop(0)
        consume(kt, et, c0)

    for hi in range(HPC):
        Eacc = ac.tile([128, 512], BF16, tag="ea", bufs=2, name=f"ea{qb}{hi}")
        pu = psP.tile([128, 512], F32, tag="pu", bufs=3, name=f"pu{qb}{hi}")
        consume = make_consume(hi, Eacc, pu)
        for kt in range(nkt):
            dj = kt - 4 * qb
            c0 = max(dj, 0) * 128
            pk = psP.tile([128, 512], F32, tag="pk", bufs=3,
                          name=f"pk{qb}{hi}{kt}")
            nc.tensor.matmul(
                pk[:, c0:], lhsT=kT_t[:, hi, kt * 128:(kt + 1) * 128],
                rhs=qTc[:, hi, c0:], start=True, stop=True)
            et = eb.tile([128, 512], BF16, tag="e", bufs=depth + 1,
                         name=f"et{qb}{hi}{kt}")
            nc.scalar.activation(
                et[:, c0:], pk[:, c0:], mybir.ActivationFunctionType.Exp,
                scale=INV_SQRT_D)
            if dj >= 0:
                # only the 128x128 diagonal block is partially masked
                nc.vector.tensor_tensor(
                    out=et[:, c0:c0 + 128], in0=et[:, c0:c0 + 128],
                    in1=msk_t[:], op=MULT)
            pend.append((consume, kt, et, c0))
            if len(pend) > depth:
                drain_one()
    while pend:
        drain_one()


def _out_proj_block(nc, qb, agt, wo_t, out, tco, psP):
    """Output projection for s-chunk qb from the gathered heads."""
    for st in range(4):
        po = psP.tile([128, OSL], F32, tag="pq", bufs=2, name=f"po{qb}{st}")
        for i in range(NKT):
            nc.tensor.matmul(
                po[:], lhsT=agt[:, i, st * 128:(st + 1) * 128],
                rhs=wo_t[:, i, :], start=(i == 0), stop=(i == NKT - 1))
        ot = tco.tile([128, OSL], BF16, tag="ot", bufs=4, name=f"ot{qb}{st}")
        nc.scalar.copy(ot[:], po[:])
        r0 = qb * 512 + st * 128
        nc.sync.dma_start(out=out[r0:r0 + 128, :], in_=ot[:])


def _build():
    nc = bacc.Bacc("TRN2", target_bir_lowering=False, debug=False,
                   num_devices=NCORES)

    # all DRAM operands pre-permuted host-side to [128, tile, cols]
    xT = nc.declare_dram_parameter("xT", [128, NKT, S], BF16, isOutput=False)
    wqT = nc.declare_dram_parameter("wqT", [128, NKT, HD], BF16, isOutput=False)
    wkT = nc.declare_dram_parameter("wkT", [128, NKT, HD], BF16, isOutput=False)
    wvT = nc.declare_dram_parameter("wvT", [128, NKT, HD], BF16, isOutput=False)
    woT = nc.declare_dram_parameter("woT", [128, NKT, OSL], BF16, isOutput=False)
    cosb = nc.declare_dram_parameter("cosb", [128, S], BF16, isOutput=False)
    sinb = nc.declare_dram_parameter("sinb", [128, S], BF16, isOutput=False)
    msk = nc.declare_dram_parameter("msk", [128, 128], BF16, isOutput=False)
    out = nc.declare_dram_parameter("out", [S, OSL], BF16, isOutput=True)

    with tile.TileContext(nc) as tc:
        with (
            tc.tile_pool(name="res", bufs=1) as res,
            tc.tile_pool(name="xa", bufs=1) as xa,
            tc.tile_pool(name="qa", bufs=1) as qa,
            tc.tile_pool(name="ta", bufs=1) as ta,
            tc.tile_pool(name="eb", bufs=1) as eb,
            tc.tile_pool(name="ac", bufs=1) as ac,
            tc.tile_pool(name="tb", bufs=1) as tb,
            tc.tile_pool(name="xc", bufs=1) as xc,
            tc.tile_pool(name="tco", bufs=1) as tco,
            tc.tile_pool(name="dram", bufs=1, space="DRAM") as dram,
            tc.tile_pool(name="psP", bufs=1, space="PSUM") as psP,
        ):
            # ---- resident tiles -------------------------------------------
            msk_t = res.tile([128, 128], BF16)
            ones_m = res.tile([128, 128], BF16)
            kT_t = res.tile([128, HPC, S], BF16)     # rope'd K^T per head
            v_t = res.tile([128, NKT, HD], BF16)     # V natural [s, dv]
            wq_t = res.tile([128, NKT, HD], BF16)
            wk_t = res.tile([128, NKT, HD], BF16)
            wv_t = res.tile([128, NKT, HD], BF16)
            wo_t = res.tile([128, NKT, OSL], BF16)
            cos_t = res.tile([128, S], BF16)
            sin_t = res.tile([128, S], BF16)

            xt_tiles = {
                t: xa.tile([128, NKT, SBW], BF16, tag="xt", bufs=2,
                           name=f"xt{t}")
                for t in range(NSB)
            }

            # prologue DMAs, in first-use order; fine 2-tile pieces so the
            # first K chain starts streaming as soon as possible.  cos/sin
            # are needed by the first rope (~16us in); wq by the Q chains
            # (~25us) -- split into pieces so the single large transfers
            # don't gate them.
            for g in range(2):
                gsl = slice(g, g + 1)
                nc.sync.dma_start(out=wk_t[:, gsl, :], in_=wkT[:, gsl, :])
                nc.sync.dma_start(out=xt_tiles[0][:, gsl, :],
                                  in_=xT[:, gsl, 0:SBW])
            nc.sync.dma_start(out=cos_t[:], in_=cosb[:])
            nc.sync.dma_start(out=sin_t[:], in_=sinb[:])
            for g in range(1, 8):
                gsl = slice(g * 2, (g + 1) * 2)
                nc.sync.dma_start(out=wk_t[:, gsl, :], in_=wkT[:, gsl, :])
                nc.sync.dma_start(out=xt_tiles[0][:, gsl, :],
                                  in_=xT[:, gsl, 0:SBW])
            for g in range(4):
                gsl = slice(g * 4, (g + 1) * 4)
                nc.sync.dma_start(out=wq_t[:, gsl, :], in_=wqT[:, gsl, :])
            nc.sync.dma_start(out=msk_t[:], in_=msk[:])
            for g in range(2):
                gsl = slice(g * 8, (g + 1) * 8)
                nc.sync.dma_start(out=wv_t[:, gsl, :], in_=wvT[:, gsl, :])
            nc.sync.dma_start(out=wo_t[:], in_=woT[:])
            nc.vector.memset(ones_m[:], 1.0)

            def stage(t, ao_c):
                ag_in = dram.tile([128, HPC, SBW], BF16, tag="agi", bufs=2,
                                  name=f"agi{t}")
                nc.sync.dma_start(out=ag_in[:], in_=ao_c[:])
                ag_out = dram.tile([4, 128, HPC, SBW], BF16, tag="ago",
                                   bufs=2, name=f"ago{t}")
                nc.gpsimd.collective_compute(
                    "AllGather",
                    mybir.AluOpType.bypass,
                    ins=[ag_in.opt()],
                    outs=[ag_out.opt()],
                    replica_groups=GROUPS,
                )
                return ag_out

            def stage_hi(t, hi, ao_c):
                # per-instance gather: lets the last chunk's collective
                # start while attention on the remaining heads still runs
                ag_in = dram.tile([128, SBW], BF16, tag="agih", bufs=4,
                                  name=f"agih{t}{hi}")
                nc.sync.dma_start(out=ag_in[:], in_=ao_c[:, hi, :])
                ag_out = dram.tile([4, 128, SBW], BF16, tag="agoh", bufs=4,
                                   name=f"agoh{t}{hi}")
                nc.gpsimd.collective_compute(
                    "AllGather",
                    mybir.AluOpType.bypass,
                    ins=[ag_in.opt()],
                    outs=[ag_out.opt()],
                    replica_groups=GROUPS,
                )
                return ag_out

            def load_agt(t, ag_out):
                agt = xc.tile([128, NKT, SBW], BF16, tag="agt", bufs=2,
                              name=f"agt{t}")
                # one DMA per source rank: 4 parallel queues (a single
                # transposed descriptor chain serializes 2MB through one
                # queue and stalls the out-projection ~10us)
                for r in range(4):
                    nc.sync.dma_start(out=agt[:, r * 4:(r + 1) * 4, :],
                                      in_=ag_out[r])
                return agt

            # ---- pipeline over the 4 sequence chunks ----------------------
            # NOTE on ordering: the framework tracks all collectives with one
            # cumulative semaphore, so any ag_out read issued AFTER cc(t) in
            # program order waits for cc(t) to land.  All load_agt calls for
            # already-landed collectives must therefore be issued BEFORE the
            # next collective_compute.
            ago_map = {}
            agt_map = {}
            for t in range(NSB):
                qTc = qa.tile([128, HPC, SBW], BF16, tag="qT", bufs=2,
                              name=f"qT{t}")
                _proj_block(nc, t, xt_tiles[t], wq_t, wk_t, wv_t,
                            cos_t, sin_t, qTc, kT_t, v_t, ta, psP)
                if t < NSB - 1:
                    nc.sync.dma_start(
                        out=xt_tiles[t + 1][:],
                        in_=xT[:, :, (t + 1) * SBW:(t + 2) * SBW])
                ao_c = qa.tile([128, HPC, SBW], BF16, tag="ao", bufs=2,
                               name=f"ao{t}")
                if t >= 2:
                    agt_map[t - 2] = load_agt(t - 2, ago_map[t - 2])
                if t == NSB - 1:
                    # pre-issue before any cc(3,hi) so it only waits cc(2)
                    agt_map[t - 1] = load_agt(t - 1, ago_map[t - 1])
                    # reuses xt(2)'s SBUF (free after proj(2)) so the
                    # interleaved loads below never wait on out-proj
                    agt3 = xa.tile([128, NKT, SBW], BF16, tag="xt", bufs=2,
                                   name="agt3")

                    def load_agt3(hi, ago_hi):
                        # head hi owns dim-tile slots hi, hi+4, hi+8, hi+12
                        nc.sync.dma_start(
                            out=agt3[:, hi:NKT:4, :],
                            in_=ago_hi[:].transpose([1, 0, 2]))

                    ago3 = []
                    _att_chunk(nc, t, qTc, kT_t, v_t, ao_c, msk_t, ones_m,
                               eb, ac, tb, psP,
                               on_finish=lambda hi: ago3.append(
                                   stage_hi(t, hi, ao_c)))
                else:
                    _att_chunk(nc, t, qTc, kT_t, v_t, ao_c, msk_t, ones_m,
                               eb, ac, tb, psP, on_finish=lambda hi: None)
                    ago_map[t] = stage(t, ao_c)
                if t >= 2:
                    _out_proj_block(nc, t - 2, agt_map[t - 2], wo_t, out,
                                    tco, psP)
            load_agt3(0, ago3[0])
            load_agt3(1, ago3[1])
            load_agt3(2, ago3[2])
            load_agt3(3, ago3[3])
            _out_proj_block(nc, NSB - 2, agt_map[NSB - 2], wo_t, out, tco,
                            psP)
            _out_proj_block(nc, NSB - 1, agt3, wo_t, out, tco, psP)
    nc.compile()
    return nc


def _host_prep(x, wq, wk, wv, wo):
    """Build per-core input maps (host-side permutes + bf16 casts)."""
    bf = ml_dtypes.bfloat16

    def perm(a2d):
        # [DIM(rows=2048), C] -> [128, 16, C] with rows = i*128 + p
        c = a2d.shape[1]
        return np.ascontiguousarray(
            a2d.reshape(NKT, 128, c).transpose(1, 0, 2)).astype(bf)

    inv = 1.0 / (10000.0 ** (np.arange(0, D, 2, dtype=np.float64) / D))
    ang = np.outer(np.arange(S, dtype=np.float64), inv)         # [S, 64]
    cos = np.cos(ang).T
    sin = np.sin(ang).T
    cosb = np.repeat(cos, 2, axis=0).astype(np.float32)         # [128, S]
    sinb = np.repeat(sin, 2, axis=0).astype(np.float32)
    sinb[0::2, :] *= -1.0    # even d rows: -sin ; odd rows: +sin

    ki = np.arange(128)[:, None]
    cj = np.arange(128)[None, :]
    mskb = (ki <= cj).astype(np.float32).astype(bf)             # [128, 128]
    cosb, sinb = cosb.astype(bf), sinb.astype(bf)

    xT_b = [perm(np.ascontiguousarray(x[b].T)) for b in range(B)]

    in_maps = []
    for c in range(NCORES):
        bb, ci = c // 4, c % 4
        hrows = slice(ci * HD, (ci + 1) * HD)       # this core's 4 heads
        ocols = slice(ci * OSL, (ci + 1) * OSL)     # its output columns
        in_maps.append({
            "xT": xT_b[bb],
            "wqT": perm(np.ascontiguousarray(wq[hrows].T)),
            "wkT": perm(np.ascontiguousarray(wk[hrows].T)),
            "wvT": perm(np.ascontiguousarray(wv[hrows].T)),
            "woT": perm(np.ascontiguousarray(wo[ocols, :].T)),
            "cosb": cosb,
            "sinb": sinb,
            "msk": mskb,
        })
    return in_maps


def kernel(x, wq, wk, wv, wo):
    global LAST_RESULT, _CACHED_NC
    if _CACHED_NC is None:
        _CACHED_NC = _build()
    nc = _CACHED_NC
    in_maps = _host_prep(x, wq, wk, wv, wo)
    res = run_bass_kernel_spmd(nc, in_maps, core_ids=list(range(NCORES)))
    LAST_RESULT = res
    out = np.empty((B, S, DIM), np.float32)
    for c in range(NCORES):
        bb, ci = c // 4, c % 4
        out[bb, :, ci * OSL:(ci + 1) * OSL] = res.results[c]["out"].astype(np.float32)
    return out
